# revision 55
# baseline (speedup 1.0000x reference)
"""LlamaAttention (B=1, S=2048, H=4096, 32 heads / 8 KV heads) on 8 TRN2 NeuronCores.

Sharding: tensor-parallel over heads. Core c owns Q heads [4c, 4c+4) and KV head c
(Wq/Wk/Wv column shards, Wo row shard). Each core computes a transposed partial
output outT = (A @ Wo)^T in [H, S]; the host sums the 8 partials and transposes.

v2 design (vs v1 which ran phases serially):
  - all HBM streams in bf16 (inputs quantized on host): X^T streamed per block,
    Wq/Wk/Wv/Wo resident in SBUF. DMA drops ~115MB -> ~46MB per core.
  - attention internals stay f32: q/k after RoPE are f32r, scores psum f32,
    exp reads psum f32; probabilities (et) are bf16.
  - softmax denominators via an extra ones-stationary matmul accumulated in
    PSUM (broadcast over partitions for free), reciprocal_approx_fast + one
    DVE mul for the normalize. No GpSimd PartitionAllReduce, no 3.4us DVE
    reciprocal.
  - causal diagonal tiles emitted FIRST per head with N-trimmed moving ranges
    (start=True covers the full width on the first tile), triangular mask-mul
    only on the single 128-wide chunk that needs it.
  - O-projection computed transposed (Wo-stationary) per 512-row block and
    emitted interleaved into the NEXT block's attention as PE filler, so the
    Tensor queue never drains (the HAM clock gate re-throttles after ~3.4us
    idle, halving matmul rate).
  - PSUM: 8 banks exactly: psA[q-proj + o-proj]x4, psK[k-proj + denom]x1,
    psX[vT-proj + v-transpose + scores]x2, psOT[attn-out]x1.
"""

import numpy as np

HIDDEN = 4096
N_HEADS = 32
N_KV = 8
HD = 128
S = 2048
N_CORES = 8
HPC = N_HEADS // N_CORES          # 4 Q heads per core
DQ = HPC * HD                     # 512 q columns per core
ROPE_BASE = 10000.0
SCALE = 1.0 / float(np.sqrt(HD))

NBLK = S // 512                   # 4 sq blocks of 512
NSK = S // 128                    # 16 sk tiles of 128
KT = HIDDEN // 128                # 32 contraction tiles

_CACHE = {}


def _build():
    import concourse.bass as bass
    import concourse.tile as tile
    from concourse import bacc, mybir

    f32 = mybir.dt.float32
    f32r = mybir.dt.float32r
    bf16 = mybir.dt.bfloat16
    EXP = mybir.ActivationFunctionType.Exp
    CPY = mybir.ActivationFunctionType.Copy

    nc = bacc.Bacc("TRN2", target_bir_lowering=False, debug=False,
                   num_devices=N_CORES)

    xt_d = nc.dram_tensor("xt", [HIDDEN, S], bf16, kind="ExternalInput").ap()
    wq_d = nc.dram_tensor("wq", [HIDDEN, DQ], bf16, kind="ExternalInput").ap()
    wk_d = nc.dram_tensor("wk", [HIDDEN, HD], bf16, kind="ExternalInput").ap()
    wv_d = nc.dram_tensor("wv", [HIDDEN, HD], bf16, kind="ExternalInput").ap()
    wo_d = nc.dram_tensor("wo", [DQ, HIDDEN], bf16, kind="ExternalInput").ap()
    cos_d = nc.dram_tensor("cosT", [HD, S], f32, kind="ExternalInput").ap()
    sin_d = nc.dram_tensor("sinS", [HD, S], f32, kind="ExternalInput").ap()
    tri_d = nc.dram_tensor("tri", [128, 128], bf16, kind="ExternalInput").ap()
    one_d = nc.dram_tensor("ones", [128, 128], bf16, kind="ExternalInput").ap()
    onf_d = nc.dram_tensor("onesf", [128, 128], f32r, kind="ExternalInput").ap()
    idn_d = nc.dram_tensor("ident", [128, 128], bf16, kind="ExternalInput").ap()
    out_d = nc.dram_tensor("out", [HIDDEN, S], bf16, kind="ExternalOutput").ap()

    xt_r = xt_d.rearrange("(kt p) s -> p kt s", p=128)
    wq_r = wq_d.rearrange("(kt p) m -> p kt m", p=128)
    wk_r = wk_d.rearrange("(kt p) m -> p kt m", p=128)
    wv_r = wv_d.rearrange("(kt p) m -> p kt m", p=128)
    wo_r = wo_d.rearrange("(hh p) n -> p hh n", p=128)
    out_r = out_d.rearrange("(t p) s -> p t s", p=128)

    with tile.TileContext(nc) as tc:
        from contextlib import ExitStack
        with ExitStack() as ctx:
            ep = ctx.enter_context
            consts = ep(tc.tile_pool(name="consts", bufs=1))
            main = ep(tc.tile_pool(name="main", bufs=1))
            xt_pool = ep(tc.tile_pool(name="xtp", bufs=6))
            et_pool = ep(tc.tile_pool(name="etp", bufs=4))
            sbc_pool = ep(tc.tile_pool(name="sbcp", bufs=1))
            tmp_pool = ep(tc.tile_pool(name="tmpp", bufs=2))
            qsp_pool = ep(tc.tile_pool(name="qspp", bufs=4))
            vts_pool = ep(tc.tile_pool(name="vtsp", bufs=2))
            osb_pool = ep(tc.tile_pool(name="osbp", bufs=2))
            ps_a = ep(tc.tile_pool(name="psa", bufs=4, space="PSUM"))
            ps_k = ep(tc.tile_pool(name="psk", bufs=1, space="PSUM"))
            ps_x = ep(tc.tile_pool(name="psx", bufs=2, space="PSUM"))
            ps_ot = ep(tc.tile_pool(name="psot", bufs=1, space="PSUM"))

            # resident weights / constants
            wq_sb = consts.tile([128, KT, DQ], bf16)
            wk_sb = consts.tile([128, KT, HD], bf16)
            wv_sb = consts.tile([128, KT, HD], bf16)
            wo_sb = consts.tile([128, HPC, HIDDEN], bf16)
            cosT = consts.tile([HD, S], f32)
            sinS = consts.tile([HD, S], f32)
            tri = consts.tile([128, 128], bf16)
            ones = consts.tile([128, 128], bf16)
            onesf = consts.tile([128, 128], f32r)
            ident = consts.tile([128, 128], bf16)
            sum_pool = ep(tc.tile_pool(name="sump", bufs=2))

            # long-lived activations
            qt = [main.tile([128, S], f32r, tag=f"qt{h}", name=f"qt{h}")
                  for h in range(HPC)]
            kt = main.tile([128, S], f32r)
            v_sb = main.tile([128, NSK, 128], bf16)
            at = [main.tile([128, S], bf16, tag=f"at{h}", name=f"at{h}")
                  for h in range(HPC)]

            def rope(eng, src, dst, blk):
                """dst[:, blk*512:+512] = rope(src); src is a PSUM tile (the
                half-partition reads are exempt from the same-base-partition
                rule only when one input is PSUM)."""
                lo = blk * 512
                sl = slice(lo, lo + 512)
                t = tmp_pool.tile([128, 512], f32, tag="ropetmp", name="ropetmp")
                eng.tensor_mul(t[0:64, :], src[64:128, :], sinS[0:64, sl])
                eng.tensor_mul(t[64:128, :], src[0:64, :], sinS[64:128, sl])
                eng.tensor_mul(dst[:, sl], src[:], cosT[:, sl])
                eng.tensor_add(dst[:, sl], dst[:, sl], t[:])

            def rope_sb(eng, q_sb, q_sw, dst, blk):
                """rope from SBUF: q_sb straight, q_sw partition-swapped copy
                (rows 0:64 = src[64:128], rows 64:128 = src[0:64]) so every
                two-SBUF-input op is full-partition with matching base."""
                lo = blk * 512
                sl = slice(lo, lo + 512)
                t = tmp_pool.tile([128, 512], f32, tag="ropetmp", name="ropetmp")
                eng.tensor_mul(t[:], q_sw[:], sinS[:, sl])
                eng.tensor_mul(dst[:, sl], q_sb[:], cosT[:, sl])
                eng.tensor_add(dst[:, sl], dst[:, sl], t[:])

            def proj_block(blk):
                """QKV projections for sq block `blk` (bf16 weights/x)."""
                lo = blk * 512
                first = (blk == 0)
                vt_ps = ps_x.tile([128, 512], f32, tag="psX", name="vtps")
                q_ps = [ps_a.tile([128, 512], f32, tag="psA", name=f"qps{h}")
                        for h in range(HPC)]
                k_ps = ps_k.tile([128, 512], f32, tag="psK", name="kps")
                for k in range(KT):
                    if k % 4 == 0:
                        if first:
                            gq = slice(k, k + 4)
                            nc.sync.dma_start(out=wq_sb[:, gq, :],
                                              in_=wq_r[:, gq, :])
                        x4 = xt_pool.tile([128, 4, 512], bf16, tag="xt",
                                          name="xt", bufs=2)
                        nc.sync.dma_start(
                            out=x4, in_=xt_r[:, k:k + 4, lo:lo + 512])
                        if first and k % 8 == 0:
                            g = slice(k, k + 8)
                            nc.sync.dma_start(out=wk_sb[:, g, :],
                                              in_=wk_r[:, g, :])
                            nc.sync.dma_start(out=wv_sb[:, g, :],
                                              in_=wv_r[:, g, :])
                    st = (k == 0)
                    sp = (k == KT - 1)
                    x_t = x4[:, k % 4, :]
                    for h in range(HPC):
                        nc.tensor.matmul(q_ps[h][:],
                                         wq_sb[:, k, h * 128:(h + 1) * 128],
                                         x_t, start=st, stop=sp)
                    nc.tensor.matmul(k_ps[:], wk_sb[:, k, :], x_t,
                                     start=st, stop=sp)
                    nc.tensor.matmul(vt_ps[:], wv_sb[:, k, :], x_t,
                                     start=st, stop=sp)
                if first:
                    nc.sync.dma_start(out=cosT, in_=cos_d)
                    nc.sync.dma_start(out=sinS, in_=sin_d)
                    nc.sync.dma_start(out=ident, in_=idn_d)
                    nc.sync.dma_start(out=tri, in_=tri_d)
                    nc.sync.dma_start(out=ones, in_=one_d)
                    nc.sync.dma_start(out=onesf, in_=onf_d)
                    for g in range(HPC):
                        nc.sync.dma_start(out=wo_sb[:, g, :], in_=wo_r[:, g, :])

                # k rope straight from psum on DVE (frees k_ps for denoms)
                rope(nc.vector, k_ps, kt, blk)
                # q: psum -> SBUF straight + partition-swapped copies on
                # Scalar.  The 4 straight copies go FIRST so all psA banks
                # free quickly (the lead o-proj groups FIFO-wait on them),
                # then the swapped halves feeding the rope math (q0 on DVE --
                # needed first -- then q1-3 on GpSimd).
                vt_sb = vts_pool.tile([128, 512], bf16, tag="vtsb", name="vtsb")
                nc.scalar.activation(vt_sb[:], vt_ps[:], CPY)
                qsp, qsw = [], []
                for h in range(HPC):
                    q_sb = qsp_pool.tile([128, 512], f32r, tag="qsp",
                                         name=f"qsp{h}")
                    nc.scalar.activation(q_sb[:], q_ps[h][:], CPY)
                    qsp.append(q_sb)
                for h in range(HPC):
                    q_sw = qsp_pool.tile([128, 512], f32r, tag="qsw",
                                         name=f"qsw{h}", bufs=2)
                    nc.scalar.activation(q_sw[0:64, :], q_ps[h][64:128, :], CPY)
                    nc.scalar.activation(q_sw[64:128, :], q_ps[h][0:64, :], CPY)
                    qsw.append(q_sw)
                # q0 on DVE (h0 needs it first), q1 on GpSimd; q2/q3 ropes
                # are deferred into the head stream so GpSimd's serial rope
                # backlog doesn't delay h0/h1's column sums (whose lag stalls
                # the bcast -> norm -> ot-bank handoff on the PE)
                rope_sb(nc.vector, qsp[0], qsw[0], qt[0], blk)
                rope_sb(nc.gpsimd, qsp[1], qsw[1], qt[1], blk)
                pend = [(qsp[2], qsw[2], 2), (qsp[3], qsw[3], 3)]
                return vt_sb, pend

            def vtrans(vt_sb, blk):
                """4 PE transposes: VT [hd, seq] -> V natural tiles."""
                for t in range(4):
                    vp = ps_x.tile([128, 128], bf16, tag="psX", name="vp")
                    nc.tensor.transpose(vp[:], vt_sb[:, t * 128:(t + 1) * 128],
                                        ident[:])
                    nc.vector.tensor_copy(v_sb[:, blk * 4 + t, :], vp[:])

            def attn_head_core(blk, h):
                """pt/exp/mask/ot chain for one head.  blk 0 also runs the
                denominators as ones-stationary matmuls (PE idles there);
                blk>=1 accumulates column sums on DVE/GpSimd instead and
                returns (sumA, sumB) for the later broadcast matmuls."""
                lo = blk * 512
                # diagonal sk tiles first (start=True on off0 covers the full
                # 512 cols), then the full tiles below the diagonal.
                seq = [4 * blk + off for off in range(4)] + list(range(4 * blk))
                on_pe = (blk == 0)
                ot_ps = ps_ot.tile([128, 512], f32, tag="psOT", name="otps")
                if on_pe:
                    dn_ps = ps_k.tile([128, 512], f32, tag="psK", name="dnps")
                    sumA = sumB = None
                else:
                    dn_ps = None
                    sumA = sum_pool.tile([128, 512], f32r, tag="sumA",
                                         name="sumA")
                    sumB = sum_pool.tile([128, 512], f32r, tag="sumB",
                                         name="sumB")
                nj = len(seq)
                na = nb = 0
                for j, i in enumerate(seq):
                    off = i - 4 * blk
                    col0 = off * 128 if off >= 0 else 0
                    st = (j == 0)
                    sp = (j == nj - 1)
                    pt = ps_x.tile([128, 512], f32, tag="psX", name="pt")
                    nc.tensor.matmul(pt[:, col0:512],
                                     kt[:, i * 128:(i + 1) * 128],
                                     qt[h][:, lo + col0:lo + 512],
                                     start=True, stop=True)
                    et = et_pool.tile([128, 512], bf16, tag="et", name="et")
                    nc.scalar.activation(et[:, col0:512], pt[:, col0:512],
                                         EXP, scale=SCALE)
                    if off >= 0:
                        nc.vector.tensor_mul(et[:, col0:col0 + 128],
                                             et[:, col0:col0 + 128], tri[:])
                    if on_pe:
                        nc.tensor.matmul(dn_ps[:, col0:512], ones[:],
                                         et[:, col0:512], start=st, stop=sp)
                    else:
                        # diagonals + odd fulls on DVE, even fulls on GpSimd
                        # (GpSimd elementwise is ~1.5x slower)
                        to_b = (off < 0) and ((i % 2) == 0)
                        if to_b:
                            eng, s, first = nc.gpsimd, sumB, (nb == 0)
                            nb += 1
                        else:
                            eng, s, first = nc.vector, sumA, (na == 0)
                            na += 1
                        if first:
                            eng.tensor_copy(s[:, col0:512], et[:, col0:512])
                        else:
                            eng.tensor_add(s[:, col0:512], s[:, col0:512],
                                           et[:, col0:512])
                    nc.tensor.matmul(ot_ps[:, col0:512], v_sb[:, i, :],
                                     et[:, col0:512], start=st, stop=sp)
                return ot_ps, dn_ps, sumA, sumB

            def attn_bcast(sumA, sumB):
                """partition-reduce+broadcast the two column-sum tiles."""
                dn_ps = ps_k.tile([128, 512], f32, tag="psK", name="dnps")
                nc.tensor.matmul(dn_ps[:], onesf[:], sumA[:],
                                 start=True, stop=False)
                nc.tensor.matmul(dn_ps[:], onesf[:], sumB[:],
                                 start=False, stop=True)
                return dn_ps

            def attn_epilogue(blk, h, ot_ps, dn_ps):
                lo = blk * 512
                sbc = sbc_pool.tile([128, 512], f32, tag="sbc", name="sbc")
                nc.vector.reciprocal_approx_fast(sbc[:], dn_ps[:])
                nc.vector.tensor_mul(at[h][:, lo:lo + 512], ot_ps[:], sbc[:])

            def oproj_group(blk, g):
                """outT rows [g*512, (g+1)*512) for seq chunk blk (g in 0..7)."""
                lo = blk * 512
                osb = osb_pool.tile([128, 4, 512], bf16, tag="osb", name="osb")
                g4 = g * 4
                for t in range(4):
                    nt = g4 + t
                    o_ps = ps_a.tile([128, 512], f32, tag="psA", name="ops")
                    for h in range(HPC):
                        nc.tensor.matmul(
                            o_ps[:],
                            wo_sb[:, h, nt * 128:(nt + 1) * 128],
                            at[h][:, lo:lo + 512],
                            start=(h == 0), stop=(h == HPC - 1))
                    if t < 2:
                        nc.scalar.activation(osb[:, t, :], o_ps[:], CPY)
                    else:
                        nc.vector.tensor_copy(osb[:, t, :], o_ps[:])
                nc.sync.dma_start(out=out_r[:, g4:g4 + 4, lo:lo + 512],
                                  in_=osb[:])

            # -------- schedule --------
            # o-proj groups of block b-1 are spread across cycle b as PE
            # filler: 2 after proj (cover the RoPE drain into h0), 2 after
            # h0 and h1, 1 after h2 and h3.  Each head's denominator
            # broadcast matmuls go after its filler groups so they never
            # stall the PE waiting on the DVE/GpSimd column sums.
            slot_groups = [(1,), (2, 3), (4, 5), (6,), (7,)]
            for blk in range(NBLK):
                if blk >= 1:
                    # group 0 leads the cycle BEFORE the k-loop: it fills the
                    # previous block's h3-epilogue window, and by running
                    # first in the psA FIFO it frees g1 from waiting on the
                    # q-copy chain after the k-loop
                    oproj_group(blk - 1, 0)
                vt_sb, pend = proj_block(blk)
                if blk >= 1:
                    for g in slot_groups[0]:
                        oproj_group(blk - 1, g)
                vtrans(vt_sb, blk)
                for h in range(HPC):
                    core = attn_head_core(blk, h)
                    ot_ps, dn_ps, sumA, sumB = core
                    if h < 2:
                        q_sb, q_sw, hh = pend[h]
                        rope_sb(nc.gpsimd, q_sb, q_sw, qt[hh], blk)
                    if blk == 0:
                        attn_epilogue(blk, h, ot_ps, dn_ps)
                    else:
                        for g in slot_groups[1 + h]:
                            oproj_group(blk - 1, g)
                        dn_ps = attn_bcast(sumA, sumB)
                        attn_epilogue(blk, h, ot_ps, dn_ps)
            for g in range(8):
                oproj_group(NBLK - 1, g)

    nc.compile()
    return nc


def _host_prep(hidden_states, position_ids, Wq, Wk, Wv, Wo):
    import ml_dtypes
    bf16 = ml_dtypes.bfloat16

    X = np.asarray(hidden_states, dtype=np.float32).reshape(S, HIDDEN)
    XT = np.ascontiguousarray(X.T).astype(bf16)

    pos = np.asarray(position_ids).reshape(-1)[:S].astype(np.float32)
    inv = (1.0 / (ROPE_BASE ** (np.arange(0, HD, 2, dtype=np.float32) / HD))
           ).astype(np.float32)
    freqs = pos[:, None] * inv[None, :]              # [S, 64]
    cos_h = np.cos(freqs).astype(np.float32)         # [S, 64] (= both halves)
    sin_h = np.sin(freqs).astype(np.float32)
    cosT = np.ascontiguousarray(np.concatenate([cos_h, cos_h], axis=1).T)
    sinT = np.concatenate([sin_h, sin_h], axis=1).T
    sinS = np.ascontiguousarray(np.concatenate([-sinT[0:64], sinT[64:128]],
                                               axis=0))

    tri = (np.arange(128)[:, None] <= np.arange(128)[None, :]).astype(bf16)
    ones = np.ones((128, 128), dtype=bf16)
    onesf = np.ones((128, 128), dtype=np.float32)
    ident = np.eye(128, dtype=np.float32).astype(bf16)

    Wq = np.asarray(Wq, dtype=np.float32)
    Wk = np.asarray(Wk, dtype=np.float32)
    Wv = np.asarray(Wv, dtype=np.float32)
    Wo = np.asarray(Wo, dtype=np.float32)

    in_maps = []
    for c in range(N_CORES):
        in_maps.append({
            "xt": XT,
            "wq": np.ascontiguousarray(Wq[:, c * DQ:(c + 1) * DQ]).astype(bf16),
            "wk": np.ascontiguousarray(Wk[:, c * HD:(c + 1) * HD]).astype(bf16),
            "wv": np.ascontiguousarray(Wv[:, c * HD:(c + 1) * HD]).astype(bf16),
            "wo": np.ascontiguousarray(Wo[c * DQ:(c + 1) * DQ, :]).astype(bf16),
            "cosT": cosT,
            "sinS": sinS,
            "tri": tri,
            "ones": ones,
            "onesf": onesf,
            "ident": ident,
        })
    return in_maps


def kernel(hidden_states, position_ids, Wq, Wk, Wv, Wo, _run_opts=None):
    from concourse.bass_utils import run_bass_kernel_spmd

    if "nc" not in _CACHE:
        _CACHE["nc"] = _build()
    nc = _CACHE["nc"]

    in_maps = _host_prep(hidden_states, position_ids, Wq, Wk, Wv, Wo)
    opts = dict(_run_opts or {})
    res = run_bass_kernel_spmd(nc, in_maps, core_ids=list(range(N_CORES)), **opts)
    _CACHE["last_result"] = res

    outT = np.zeros((HIDDEN, S), dtype=np.float64)
    for c in range(N_CORES):
        outT += res.results[c]["out"].astype(np.float32)
    return outT.T.astype(np.float32).reshape(1, S, HIDDEN)


# revision 58
# speedup vs baseline: 1.0054x; 1.0054x over previous
"""LlamaAttention (B=1, S=2048, H=4096, 32 heads / 8 KV heads) on 8 TRN2 NeuronCores.

Sharding: tensor-parallel over heads. Core c owns Q heads [4c, 4c+4) and KV head c
(Wq/Wk/Wv column shards, Wo row shard). Each core computes a transposed partial
output outT = (A @ Wo)^T in [H, S]; the host sums the 8 partials and transposes.

v2 design (vs v1 which ran phases serially):
  - all HBM streams in bf16 (inputs quantized on host): X^T streamed per block,
    Wq/Wk/Wv/Wo resident in SBUF. DMA drops ~115MB -> ~46MB per core.
  - attention internals stay f32: q/k after RoPE are f32r, scores psum f32,
    exp reads psum f32; probabilities (et) are bf16.
  - softmax denominators via an extra ones-stationary matmul accumulated in
    PSUM (broadcast over partitions for free), reciprocal_approx_fast + one
    DVE mul for the normalize. No GpSimd PartitionAllReduce, no 3.4us DVE
    reciprocal.
  - causal diagonal tiles emitted FIRST per head with N-trimmed moving ranges
    (start=True covers the full width on the first tile), triangular mask-mul
    only on the single 128-wide chunk that needs it.
  - O-projection computed transposed (Wo-stationary) per 512-row block and
    emitted interleaved into the NEXT block's attention as PE filler, so the
    Tensor queue never drains (the HAM clock gate re-throttles after ~3.4us
    idle, halving matmul rate).
  - PSUM: 8 banks exactly: psA[q-proj + o-proj]x4, psK[k-proj + denom]x1,
    psX[vT-proj + v-transpose + scores]x2, psOT[attn-out]x1.
"""

import numpy as np

HIDDEN = 4096
N_HEADS = 32
N_KV = 8
HD = 128
S = 2048
N_CORES = 8
HPC = N_HEADS // N_CORES          # 4 Q heads per core
DQ = HPC * HD                     # 512 q columns per core
ROPE_BASE = 10000.0
SCALE = 1.0 / float(np.sqrt(HD))

NBLK = S // 512                   # 4 sq blocks of 512
NSK = S // 128                    # 16 sk tiles of 128
KT = HIDDEN // 128                # 32 contraction tiles

_CACHE = {}


def _build():
    import concourse.bass as bass
    import concourse.tile as tile
    from concourse import bacc, mybir

    f32 = mybir.dt.float32
    f32r = mybir.dt.float32r
    bf16 = mybir.dt.bfloat16
    EXP = mybir.ActivationFunctionType.Exp
    CPY = mybir.ActivationFunctionType.Copy

    nc = bacc.Bacc("TRN2", target_bir_lowering=False, debug=False,
                   num_devices=N_CORES)

    xt_d = nc.dram_tensor("xt", [HIDDEN, S], bf16, kind="ExternalInput").ap()
    wq_d = nc.dram_tensor("wq", [HIDDEN, DQ], bf16, kind="ExternalInput").ap()
    wk_d = nc.dram_tensor("wk", [HIDDEN, HD], bf16, kind="ExternalInput").ap()
    wv_d = nc.dram_tensor("wv", [HIDDEN, HD], bf16, kind="ExternalInput").ap()
    wo_d = nc.dram_tensor("wo", [DQ, HIDDEN], bf16, kind="ExternalInput").ap()
    cos_d = nc.dram_tensor("cosT", [HD, S], f32, kind="ExternalInput").ap()
    sin_d = nc.dram_tensor("sinS", [HD, S], f32, kind="ExternalInput").ap()
    tri_d = nc.dram_tensor("tri", [128, 128], bf16, kind="ExternalInput").ap()
    one_d = nc.dram_tensor("ones", [128, 128], bf16, kind="ExternalInput").ap()
    onf_d = nc.dram_tensor("onesf", [128, 128], f32r, kind="ExternalInput").ap()
    idn_d = nc.dram_tensor("ident", [128, 128], bf16, kind="ExternalInput").ap()
    out_d = nc.dram_tensor("out", [HIDDEN, S], bf16, kind="ExternalOutput").ap()

    xt_r = xt_d.rearrange("(kt p) s -> p kt s", p=128)
    wq_r = wq_d.rearrange("(kt p) m -> p kt m", p=128)
    wk_r = wk_d.rearrange("(kt p) m -> p kt m", p=128)
    wv_r = wv_d.rearrange("(kt p) m -> p kt m", p=128)
    wo_r = wo_d.rearrange("(hh p) n -> p hh n", p=128)
    out_r = out_d.rearrange("(t p) s -> p t s", p=128)

    with tile.TileContext(nc) as tc:
        from contextlib import ExitStack
        with ExitStack() as ctx:
            ep = ctx.enter_context
            consts = ep(tc.tile_pool(name="consts", bufs=1))
            main = ep(tc.tile_pool(name="main", bufs=1))
            xt_pool = ep(tc.tile_pool(name="xtp", bufs=6))
            et_pool = ep(tc.tile_pool(name="etp", bufs=4))
            sbc_pool = ep(tc.tile_pool(name="sbcp", bufs=1))
            tmp_pool = ep(tc.tile_pool(name="tmpp", bufs=2))
            qsp_pool = ep(tc.tile_pool(name="qspp", bufs=4))
            vts_pool = ep(tc.tile_pool(name="vtsp", bufs=2))
            osb_pool = ep(tc.tile_pool(name="osbp", bufs=2))
            ps_a = ep(tc.tile_pool(name="psa", bufs=4, space="PSUM"))
            ps_k = ep(tc.tile_pool(name="psk", bufs=1, space="PSUM"))
            ps_x = ep(tc.tile_pool(name="psx", bufs=2, space="PSUM"))
            ps_ot = ep(tc.tile_pool(name="psot", bufs=1, space="PSUM"))

            # resident weights / constants
            wq_sb = consts.tile([128, KT, DQ], bf16)
            wk_sb = consts.tile([128, KT, HD], bf16)
            wv_sb = consts.tile([128, KT, HD], bf16)
            wo_sb = consts.tile([128, HPC, HIDDEN], bf16)
            cosT = consts.tile([HD, S], f32)
            sinS = consts.tile([HD, S], f32)
            tri = consts.tile([128, 128], bf16)
            ones = consts.tile([128, 128], bf16)
            onesf = consts.tile([128, 128], f32r)
            ident = consts.tile([128, 128], bf16)
            sum_pool = ep(tc.tile_pool(name="sump", bufs=2))

            # long-lived activations
            qt = [main.tile([128, S], f32r, tag=f"qt{h}", name=f"qt{h}")
                  for h in range(HPC)]
            kt = main.tile([128, S], f32r)
            v_sb = main.tile([128, NSK, 128], bf16)
            at = [main.tile([128, S], bf16, tag=f"at{h}", name=f"at{h}")
                  for h in range(HPC)]

            def rope(eng, src, dst, blk):
                """dst[:, blk*512:+512] = rope(src); src is a PSUM tile (the
                half-partition reads are exempt from the same-base-partition
                rule only when one input is PSUM)."""
                lo = blk * 512
                sl = slice(lo, lo + 512)
                t = tmp_pool.tile([128, 512], f32, tag="ropetmp", name="ropetmp")
                eng.tensor_mul(t[0:64, :], src[64:128, :], sinS[0:64, sl])
                eng.tensor_mul(t[64:128, :], src[0:64, :], sinS[64:128, sl])
                eng.tensor_mul(dst[:, sl], src[:], cosT[:, sl])
                eng.tensor_add(dst[:, sl], dst[:, sl], t[:])

            def rope_sb(eng, q_sb, q_sw, dst, blk):
                """rope from SBUF: q_sb straight, q_sw partition-swapped copy
                (rows 0:64 = src[64:128], rows 64:128 = src[0:64]) so every
                two-SBUF-input op is full-partition with matching base."""
                lo = blk * 512
                sl = slice(lo, lo + 512)
                t = tmp_pool.tile([128, 512], f32, tag="ropetmp", name="ropetmp")
                eng.tensor_mul(t[:], q_sw[:], sinS[:, sl])
                eng.tensor_mul(dst[:, sl], q_sb[:], cosT[:, sl])
                eng.tensor_add(dst[:, sl], dst[:, sl], t[:])

            def proj_block(blk):
                """QKV projections for sq block `blk` (bf16 weights/x)."""
                lo = blk * 512
                first = (blk == 0)
                vt_ps = ps_x.tile([128, 512], f32, tag="psX", name="vtps")
                q_ps = [ps_a.tile([128, 512], f32, tag="psA", name=f"qps{h}")
                        for h in range(HPC)]
                k_ps = ps_k.tile([128, 512], f32, tag="psK", name="kps")
                for k in range(KT):
                    if k % 4 == 0:
                        if first:
                            gq = slice(k, k + 4)
                            nc.sync.dma_start(out=wq_sb[:, gq, :],
                                              in_=wq_r[:, gq, :])
                        x4 = xt_pool.tile([128, 4, 512], bf16, tag="xt",
                                          name="xt", bufs=2)
                        nc.sync.dma_start(
                            out=x4, in_=xt_r[:, k:k + 4, lo:lo + 512])
                        if first and k % 8 == 0:
                            g = slice(k, k + 8)
                            nc.sync.dma_start(out=wk_sb[:, g, :],
                                              in_=wk_r[:, g, :])
                            nc.sync.dma_start(out=wv_sb[:, g, :],
                                              in_=wv_r[:, g, :])
                    st = (k == 0)
                    sp = (k == KT - 1)
                    x_t = x4[:, k % 4, :]
                    for h in range(HPC):
                        nc.tensor.matmul(q_ps[h][:],
                                         wq_sb[:, k, h * 128:(h + 1) * 128],
                                         x_t, start=st, stop=sp)
                    nc.tensor.matmul(k_ps[:], wk_sb[:, k, :], x_t,
                                     start=st, stop=sp)
                    nc.tensor.matmul(vt_ps[:], wv_sb[:, k, :], x_t,
                                     start=st, stop=sp)
                if first:
                    nc.sync.dma_start(out=cosT, in_=cos_d)
                    nc.sync.dma_start(out=sinS, in_=sin_d)
                    nc.sync.dma_start(out=ident, in_=idn_d)
                    nc.sync.dma_start(out=tri, in_=tri_d)
                    nc.sync.dma_start(out=ones, in_=one_d)
                    nc.sync.dma_start(out=onesf, in_=onf_d)

                # k rope straight from psum on DVE (frees k_ps for denoms)
                rope(nc.vector, k_ps, kt, blk)
                # q: psum -> SBUF straight + partition-swapped copies on
                # Scalar.  The 4 straight copies go FIRST so all psA banks
                # free quickly (the lead o-proj groups FIFO-wait on them),
                # then the swapped halves feeding the rope math (q0 on DVE --
                # needed first -- then q1-3 on GpSimd).
                vt_sb = vts_pool.tile([128, 512], bf16, tag="vtsb", name="vtsb")
                nc.scalar.activation(vt_sb[:], vt_ps[:], CPY)
                qsp, qsw = [], []
                for h in range(HPC):
                    q_sb = qsp_pool.tile([128, 512], f32r, tag="qsp",
                                         name=f"qsp{h}")
                    nc.scalar.activation(q_sb[:], q_ps[h][:], CPY)
                    qsp.append(q_sb)
                for h in range(HPC):
                    q_sw = qsp_pool.tile([128, 512], f32r, tag="qsw",
                                         name=f"qsw{h}", bufs=2)
                    nc.scalar.activation(q_sw[0:64, :], q_ps[h][64:128, :], CPY)
                    nc.scalar.activation(q_sw[64:128, :], q_ps[h][0:64, :], CPY)
                    qsw.append(q_sw)
                # q0 on DVE (h0 needs it first), q1 on GpSimd; q2/q3 ropes
                # are deferred into the head stream so GpSimd's serial rope
                # backlog doesn't delay h0/h1's column sums (whose lag stalls
                # the bcast -> norm -> ot-bank handoff on the PE)
                rope_sb(nc.vector, qsp[0], qsw[0], qt[0], blk)
                rope_sb(nc.gpsimd, qsp[1], qsw[1], qt[1], blk)
                pend = [(qsp[2], qsw[2], 2), (qsp[3], qsw[3], 3)]
                return vt_sb, pend

            def vtrans(vt_sb, blk):
                """4 PE transposes: VT [hd, seq] -> V natural tiles."""
                for t in range(4):
                    vp = ps_x.tile([128, 128], bf16, tag="psX", name="vp")
                    nc.tensor.transpose(vp[:], vt_sb[:, t * 128:(t + 1) * 128],
                                        ident[:])
                    nc.vector.tensor_copy(v_sb[:, blk * 4 + t, :], vp[:])

            def attn_head_core(blk, h):
                """pt/exp/mask/ot chain for one head.  blk 0 also runs the
                denominators as ones-stationary matmuls (PE idles there);
                blk>=1 accumulates column sums on DVE/GpSimd instead and
                returns (sumA, sumB) for the later broadcast matmuls."""
                lo = blk * 512
                # diagonal sk tiles first (start=True on off0 covers the full
                # 512 cols), then the full tiles below the diagonal.
                seq = [4 * blk + off for off in range(4)] + list(range(4 * blk))
                on_pe = (blk == 0)
                ot_ps = ps_ot.tile([128, 512], f32, tag="psOT", name="otps")
                if on_pe:
                    dn_ps = ps_k.tile([128, 512], f32, tag="psK", name="dnps")
                    sumA = sumB = None
                else:
                    dn_ps = None
                    sumA = sum_pool.tile([128, 512], f32r, tag="sumA",
                                         name="sumA")
                    sumB = sum_pool.tile([128, 512], f32r, tag="sumB",
                                         name="sumB")
                nj = len(seq)
                na = nb = 0
                for j, i in enumerate(seq):
                    off = i - 4 * blk
                    col0 = off * 128 if off >= 0 else 0
                    st = (j == 0)
                    sp = (j == nj - 1)
                    pt = ps_x.tile([128, 512], f32, tag="psX", name="pt")
                    nc.tensor.matmul(pt[:, col0:512],
                                     kt[:, i * 128:(i + 1) * 128],
                                     qt[h][:, lo + col0:lo + 512],
                                     start=True, stop=True)
                    et = et_pool.tile([128, 512], bf16, tag="et", name="et")
                    nc.scalar.activation(et[:, col0:512], pt[:, col0:512],
                                         EXP, scale=SCALE)
                    if off >= 0:
                        nc.vector.tensor_mul(et[:, col0:col0 + 128],
                                             et[:, col0:col0 + 128], tri[:])
                    if on_pe:
                        nc.tensor.matmul(dn_ps[:, col0:512], ones[:],
                                         et[:, col0:512], start=st, stop=sp)
                    else:
                        # diagonals + odd fulls on DVE, even fulls on GpSimd
                        # (GpSimd elementwise is ~1.5x slower)
                        to_b = (off < 0) and ((i % 2) == 0)
                        if to_b:
                            eng, s, first = nc.gpsimd, sumB, (nb == 0)
                            nb += 1
                        else:
                            eng, s, first = nc.vector, sumA, (na == 0)
                            na += 1
                        if first:
                            eng.tensor_copy(s[:, col0:512], et[:, col0:512])
                        else:
                            eng.tensor_add(s[:, col0:512], s[:, col0:512],
                                           et[:, col0:512])
                    nc.tensor.matmul(ot_ps[:, col0:512], v_sb[:, i, :],
                                     et[:, col0:512], start=st, stop=sp)
                return ot_ps, dn_ps, sumA, sumB

            def attn_bcast(sumA, sumB):
                """partition-reduce+broadcast the two column-sum tiles."""
                dn_ps = ps_k.tile([128, 512], f32, tag="psK", name="dnps")
                nc.tensor.matmul(dn_ps[:], onesf[:], sumA[:],
                                 start=True, stop=False)
                nc.tensor.matmul(dn_ps[:], onesf[:], sumB[:],
                                 start=False, stop=True)
                return dn_ps

            def attn_epilogue(blk, h, ot_ps, dn_ps):
                lo = blk * 512
                sbc = sbc_pool.tile([128, 512], f32, tag="sbc", name="sbc")
                nc.vector.reciprocal_approx_fast(sbc[:], dn_ps[:])
                nc.vector.tensor_mul(at[h][:, lo:lo + 512], ot_ps[:], sbc[:])

            def oproj_group(blk, g):
                """outT rows [g*512, (g+1)*512) for seq chunk blk (g in 0..7)."""
                lo = blk * 512
                osb = osb_pool.tile([128, 4, 512], bf16, tag="osb", name="osb")
                g4 = g * 4
                for t in range(4):
                    nt = g4 + t
                    o_ps = ps_a.tile([128, 512], f32, tag="psA", name="ops")
                    for h in range(HPC):
                        nc.tensor.matmul(
                            o_ps[:],
                            wo_sb[:, h, nt * 128:(nt + 1) * 128],
                            at[h][:, lo:lo + 512],
                            start=(h == 0), stop=(h == HPC - 1))
                    if t < 2:
                        nc.scalar.activation(osb[:, t, :], o_ps[:], CPY)
                    else:
                        nc.vector.tensor_copy(osb[:, t, :], o_ps[:])
                nc.sync.dma_start(out=out_r[:, g4:g4 + 4, lo:lo + 512],
                                  in_=osb[:])

            # -------- schedule --------
            # o-proj groups of block b-1 are spread across cycle b as PE
            # filler: 2 after proj (cover the RoPE drain into h0), 2 after
            # h0 and h1, 1 after h2 and h3.  Each head's denominator
            # broadcast matmuls go after its filler groups so they never
            # stall the PE waiting on the DVE/GpSimd column sums.
            slot_groups = [(0, 1), (2, 3), (4, 5), (6,), (7,)]
            for blk in range(NBLK):
                vt_sb, pend = proj_block(blk)
                if blk >= 1:
                    for g in slot_groups[0]:
                        oproj_group(blk - 1, g)
                vtrans(vt_sb, blk)
                for h in range(HPC):
                    core = attn_head_core(blk, h)
                    ot_ps, dn_ps, sumA, sumB = core
                    if h < 2:
                        q_sb, q_sw, hh = pend[h]
                        rope_sb(nc.gpsimd, q_sb, q_sw, qt[hh], blk)
                    if blk == 0:
                        attn_epilogue(blk, h, ot_ps, dn_ps)
                        # resident Wo load spread across cycle-0's attention
                        # (the k-loop window's DMA is saturated by the next
                        # block's x prefetch); statements identical to the
                        # old consts-tail load, just emitted later
                        nc.sync.dma_start(out=wo_sb[:, h, :], in_=wo_r[:, h, :])
                    else:
                        for g in slot_groups[1 + h]:
                            oproj_group(blk - 1, g)
                        dn_ps = attn_bcast(sumA, sumB)
                        attn_epilogue(blk, h, ot_ps, dn_ps)
            for g in range(8):
                oproj_group(NBLK - 1, g)

    nc.compile()
    return nc


def _host_prep(hidden_states, position_ids, Wq, Wk, Wv, Wo):
    import ml_dtypes
    bf16 = ml_dtypes.bfloat16

    X = np.asarray(hidden_states, dtype=np.float32).reshape(S, HIDDEN)
    XT = np.ascontiguousarray(X.T).astype(bf16)

    pos = np.asarray(position_ids).reshape(-1)[:S].astype(np.float32)
    inv = (1.0 / (ROPE_BASE ** (np.arange(0, HD, 2, dtype=np.float32) / HD))
           ).astype(np.float32)
    freqs = pos[:, None] * inv[None, :]              # [S, 64]
    cos_h = np.cos(freqs).astype(np.float32)         # [S, 64] (= both halves)
    sin_h = np.sin(freqs).astype(np.float32)
    cosT = np.ascontiguousarray(np.concatenate([cos_h, cos_h], axis=1).T)
    sinT = np.concatenate([sin_h, sin_h], axis=1).T
    sinS = np.ascontiguousarray(np.concatenate([-sinT[0:64], sinT[64:128]],
                                               axis=0))

    tri = (np.arange(128)[:, None] <= np.arange(128)[None, :]).astype(bf16)
    ones = np.ones((128, 128), dtype=bf16)
    onesf = np.ones((128, 128), dtype=np.float32)
    ident = np.eye(128, dtype=np.float32).astype(bf16)

    Wq = np.asarray(Wq, dtype=np.float32)
    Wk = np.asarray(Wk, dtype=np.float32)
    Wv = np.asarray(Wv, dtype=np.float32)
    Wo = np.asarray(Wo, dtype=np.float32)

    in_maps = []
    for c in range(N_CORES):
        in_maps.append({
            "xt": XT,
            "wq": np.ascontiguousarray(Wq[:, c * DQ:(c + 1) * DQ]).astype(bf16),
            "wk": np.ascontiguousarray(Wk[:, c * HD:(c + 1) * HD]).astype(bf16),
            "wv": np.ascontiguousarray(Wv[:, c * HD:(c + 1) * HD]).astype(bf16),
            "wo": np.ascontiguousarray(Wo[c * DQ:(c + 1) * DQ, :]).astype(bf16),
            "cosT": cosT,
            "sinS": sinS,
            "tri": tri,
            "ones": ones,
            "onesf": onesf,
            "ident": ident,
        })
    return in_maps


def kernel(hidden_states, position_ids, Wq, Wk, Wv, Wo, _run_opts=None):
    from concourse.bass_utils import run_bass_kernel_spmd

    if "nc" not in _CACHE:
        _CACHE["nc"] = _build()
    nc = _CACHE["nc"]

    in_maps = _host_prep(hidden_states, position_ids, Wq, Wk, Wv, Wo)
    opts = dict(_run_opts or {})
    res = run_bass_kernel_spmd(nc, in_maps, core_ids=list(range(N_CORES)), **opts)
    _CACHE["last_result"] = res

    outT = np.zeros((HIDDEN, S), dtype=np.float64)
    for c in range(N_CORES):
        outT += res.results[c]["out"].astype(np.float32)
    return outT.T.astype(np.float32).reshape(1, S, HIDDEN)


# revision 60
# speedup vs baseline: 1.0153x; 1.0098x over previous
"""LlamaAttention (B=1, S=2048, H=4096, 32 heads / 8 KV heads) on 8 TRN2 NeuronCores.

Sharding: tensor-parallel over heads. Core c owns Q heads [4c, 4c+4) and KV head c
(Wq/Wk/Wv column shards, Wo row shard). Each core computes a transposed partial
output outT = (A @ Wo)^T in [H, S]; the host sums the 8 partials and transposes.

v2 design (vs v1 which ran phases serially):
  - all HBM streams in bf16 (inputs quantized on host): X^T streamed per block,
    Wq/Wk/Wv/Wo resident in SBUF. DMA drops ~115MB -> ~46MB per core.
  - attention internals stay f32: q/k after RoPE are f32r, scores psum f32,
    exp reads psum f32; probabilities (et) are bf16.
  - softmax denominators via an extra ones-stationary matmul accumulated in
    PSUM (broadcast over partitions for free), reciprocal_approx_fast + one
    DVE mul for the normalize. No GpSimd PartitionAllReduce, no 3.4us DVE
    reciprocal.
  - causal diagonal tiles emitted FIRST per head with N-trimmed moving ranges
    (start=True covers the full width on the first tile), triangular mask-mul
    only on the single 128-wide chunk that needs it.
  - O-projection computed transposed (Wo-stationary) per 512-row block and
    emitted interleaved into the NEXT block's attention as PE filler, so the
    Tensor queue never drains (the HAM clock gate re-throttles after ~3.4us
    idle, halving matmul rate).
  - PSUM: 8 banks exactly: psA[q-proj + o-proj]x4, psK[k-proj + denom]x1,
    psX[vT-proj + v-transpose + scores]x2, psOT[attn-out]x1.
"""

import numpy as np

HIDDEN = 4096
N_HEADS = 32
N_KV = 8
HD = 128
S = 2048
N_CORES = 8
HPC = N_HEADS // N_CORES          # 4 Q heads per core
DQ = HPC * HD                     # 512 q columns per core
ROPE_BASE = 10000.0
SCALE = 1.0 / float(np.sqrt(HD))

NBLK = S // 512                   # 4 sq blocks of 512
NSK = S // 128                    # 16 sk tiles of 128
KT = HIDDEN // 128                # 32 contraction tiles

_CACHE = {}


def _build():
    import concourse.bass as bass
    import concourse.tile as tile
    from concourse import bacc, mybir

    f32 = mybir.dt.float32
    f32r = mybir.dt.float32r
    bf16 = mybir.dt.bfloat16
    EXP = mybir.ActivationFunctionType.Exp
    CPY = mybir.ActivationFunctionType.Copy

    nc = bacc.Bacc("TRN2", target_bir_lowering=False, debug=False,
                   num_devices=N_CORES)

    xt_d = nc.dram_tensor("xt", [HIDDEN, S], bf16, kind="ExternalInput").ap()
    wq_d = nc.dram_tensor("wq", [HIDDEN, DQ], bf16, kind="ExternalInput").ap()
    wk_d = nc.dram_tensor("wk", [HIDDEN, HD], bf16, kind="ExternalInput").ap()
    wv_d = nc.dram_tensor("wv", [HIDDEN, HD], bf16, kind="ExternalInput").ap()
    wo_d = nc.dram_tensor("wo", [DQ, HIDDEN], bf16, kind="ExternalInput").ap()
    cos_d = nc.dram_tensor("cosT", [HD, S], f32, kind="ExternalInput").ap()
    sin_d = nc.dram_tensor("sinS", [HD, S], f32, kind="ExternalInput").ap()
    tri_d = nc.dram_tensor("tri", [128, 128], bf16, kind="ExternalInput").ap()
    one_d = nc.dram_tensor("ones", [128, 128], bf16, kind="ExternalInput").ap()
    onf_d = nc.dram_tensor("onesf", [128, 128], f32r, kind="ExternalInput").ap()
    idn_d = nc.dram_tensor("ident", [128, 128], bf16, kind="ExternalInput").ap()
    out_d = nc.dram_tensor("out", [HIDDEN, S], bf16, kind="ExternalOutput").ap()

    xt_r = xt_d.rearrange("(kt p) s -> p kt s", p=128)
    wq_r = wq_d.rearrange("(kt p) m -> p kt m", p=128)
    wk_r = wk_d.rearrange("(kt p) m -> p kt m", p=128)
    wv_r = wv_d.rearrange("(kt p) m -> p kt m", p=128)
    wo_r = wo_d.rearrange("(hh p) n -> p hh n", p=128)
    out_r = out_d.rearrange("(t p) s -> p t s", p=128)

    with tile.TileContext(nc) as tc:
        from contextlib import ExitStack
        with ExitStack() as ctx:
            ep = ctx.enter_context
            consts = ep(tc.tile_pool(name="consts", bufs=1))
            main = ep(tc.tile_pool(name="main", bufs=1))
            xt_pool = ep(tc.tile_pool(name="xtp", bufs=6))
            et_pool = ep(tc.tile_pool(name="etp", bufs=4))
            sbc_pool = ep(tc.tile_pool(name="sbcp", bufs=1))
            tmp_pool = ep(tc.tile_pool(name="tmpp", bufs=2))
            qsp_pool = ep(tc.tile_pool(name="qspp", bufs=4))
            vts_pool = ep(tc.tile_pool(name="vtsp", bufs=2))
            osb_pool = ep(tc.tile_pool(name="osbp", bufs=2))
            ps_a = ep(tc.tile_pool(name="psa", bufs=4, space="PSUM"))
            ps_k = ep(tc.tile_pool(name="psk", bufs=1, space="PSUM"))
            ps_x = ep(tc.tile_pool(name="psx", bufs=2, space="PSUM"))
            ps_ot = ep(tc.tile_pool(name="psot", bufs=1, space="PSUM"))

            # resident weights / constants
            wq_sb = consts.tile([128, KT, DQ], bf16)
            wk_sb = consts.tile([128, KT, HD], bf16)
            wv_sb = consts.tile([128, KT, HD], bf16)
            wo_sb = consts.tile([128, HPC, HIDDEN], bf16)
            cosT = consts.tile([HD, S], f32)
            sinS = consts.tile([HD, S], f32)
            tri = consts.tile([128, 128], bf16)
            ones = consts.tile([128, 128], bf16)
            onesf = consts.tile([128, 128], f32r)
            ident = consts.tile([128, 128], bf16)
            sum_pool = ep(tc.tile_pool(name="sump", bufs=2))

            # long-lived activations
            qt = [main.tile([128, S], f32r, tag=f"qt{h}", name=f"qt{h}")
                  for h in range(HPC)]
            kt = main.tile([128, S], f32r)
            v_sb = main.tile([128, NSK, 128], bf16)
            at = [main.tile([128, S], bf16, tag=f"at{h}", name=f"at{h}")
                  for h in range(HPC)]

            def rope(eng, src, dst, blk):
                """dst[:, blk*512:+512] = rope(src); src is a PSUM tile (the
                half-partition reads are exempt from the same-base-partition
                rule only when one input is PSUM)."""
                lo = blk * 512
                sl = slice(lo, lo + 512)
                t = tmp_pool.tile([128, 512], f32, tag="ropetmp", name="ropetmp")
                eng.tensor_mul(t[0:64, :], src[64:128, :], sinS[0:64, sl])
                eng.tensor_mul(t[64:128, :], src[0:64, :], sinS[64:128, sl])
                eng.tensor_mul(dst[:, sl], src[:], cosT[:, sl])
                eng.tensor_add(dst[:, sl], dst[:, sl], t[:])

            def rope_sb(eng, q_sb, q_sw, dst, blk):
                """rope from SBUF: q_sb straight, q_sw partition-swapped copy
                (rows 0:64 = src[64:128], rows 64:128 = src[0:64]) so every
                two-SBUF-input op is full-partition with matching base."""
                lo = blk * 512
                sl = slice(lo, lo + 512)
                t = tmp_pool.tile([128, 512], f32, tag="ropetmp", name="ropetmp")
                eng.tensor_mul(t[:], q_sw[:], sinS[:, sl])
                eng.tensor_mul(dst[:, sl], q_sb[:], cosT[:, sl])
                eng.tensor_add(dst[:, sl], dst[:, sl], t[:])

            def proj_block(blk):
                """QKV projections for sq block `blk` (bf16 weights/x)."""
                lo = blk * 512
                first = (blk == 0)
                vt_ps = ps_x.tile([128, 512], f32, tag="psX", name="vtps")
                q_ps = [ps_a.tile([128, 512], f32, tag="psA", name=f"qps{h}")
                        for h in range(HPC)]
                k_ps = ps_k.tile([128, 512], f32, tag="psK", name="kps")
                for k in range(KT):
                    if k % 4 == 0:
                        if first:
                            gq = slice(k, k + 4)
                            nc.sync.dma_start(out=wq_sb[:, gq, :],
                                              in_=wq_r[:, gq, :])
                        x4 = xt_pool.tile([128, 4, 512], bf16, tag="xt",
                                          name="xt", bufs=2)
                        nc.sync.dma_start(
                            out=x4, in_=xt_r[:, k:k + 4, lo:lo + 512])
                        if first and k % 8 == 0:
                            g = slice(k, k + 8)
                            nc.sync.dma_start(out=wk_sb[:, g, :],
                                              in_=wk_r[:, g, :])
                            nc.sync.dma_start(out=wv_sb[:, g, :],
                                              in_=wv_r[:, g, :])
                    st = (k == 0)
                    sp = (k == KT - 1)
                    x_t = x4[:, k % 4, :]
                    for h in range(HPC):
                        nc.tensor.matmul(q_ps[h][:],
                                         wq_sb[:, k, h * 128:(h + 1) * 128],
                                         x_t, start=st, stop=sp)
                    nc.tensor.matmul(k_ps[:], wk_sb[:, k, :], x_t,
                                     start=st, stop=sp)
                    nc.tensor.matmul(vt_ps[:], wv_sb[:, k, :], x_t,
                                     start=st, stop=sp)
                if first:
                    nc.sync.dma_start(out=cosT, in_=cos_d)
                    nc.sync.dma_start(out=sinS, in_=sin_d)
                    nc.sync.dma_start(out=ident, in_=idn_d)
                    nc.sync.dma_start(out=tri, in_=tri_d)
                    nc.sync.dma_start(out=ones, in_=one_d)
                    nc.sync.dma_start(out=onesf, in_=onf_d)
                    for g in range(HPC):
                        nc.sync.dma_start(out=wo_sb[:, g, :], in_=wo_r[:, g, :])

                # k rope straight from psum on DVE (frees k_ps for denoms)
                rope(nc.vector, k_ps, kt, blk)
                # q: psum -> SBUF straight + partition-swapped copies on
                # Scalar.  The 4 straight copies go FIRST so all psA banks
                # free quickly (the lead o-proj groups FIFO-wait on them),
                # then the swapped halves feeding the rope math (q0 on DVE --
                # needed first -- then q1-3 on GpSimd).
                vt_sb = vts_pool.tile([128, 512], bf16, tag="vtsb", name="vtsb")
                nc.scalar.activation(vt_sb[:], vt_ps[:], CPY)
                qsp, qsw = [], []
                for h in range(HPC):
                    q_sb = qsp_pool.tile([128, 512], f32r, tag="qsp",
                                         name=f"qsp{h}")
                    nc.scalar.activation(q_sb[:], q_ps[h][:], CPY)
                    qsp.append(q_sb)
                for h in range(HPC):
                    q_sw = qsp_pool.tile([128, 512], f32r, tag="qsw",
                                         name=f"qsw{h}", bufs=2)
                    nc.scalar.activation(q_sw[0:64, :], q_ps[h][64:128, :], CPY)
                    nc.scalar.activation(q_sw[64:128, :], q_ps[h][0:64, :], CPY)
                    qsw.append(q_sw)
                # q0 on DVE (h0 needs it first), q1 on GpSimd; q2/q3 ropes
                # are deferred into the head stream so GpSimd's serial rope
                # backlog doesn't delay h0/h1's column sums (whose lag stalls
                # the bcast -> norm -> ot-bank handoff on the PE)
                rope_sb(nc.vector, qsp[0], qsw[0], qt[0], blk)
                rope_sb(nc.gpsimd, qsp[1], qsw[1], qt[1], blk)
                pend = [(qsp[2], qsw[2], 2), (qsp[3], qsw[3], 3)]
                return vt_sb, pend

            def vtrans(vt_sb, blk):
                """4 PE transposes: VT [hd, seq] -> V natural tiles."""
                for t in range(4):
                    vp = ps_x.tile([128, 128], bf16, tag="psX", name="vp")
                    nc.tensor.transpose(vp[:], vt_sb[:, t * 128:(t + 1) * 128],
                                        ident[:])
                    nc.vector.tensor_copy(v_sb[:, blk * 4 + t, :], vp[:])

            def attn_head_core(blk, h):
                """pt/exp/mask/ot chain for one head.  blk 0 also runs the
                denominators as ones-stationary matmuls (PE idles there);
                blk>=1 accumulates column sums on DVE/GpSimd instead and
                returns (sumA, sumB) for the later broadcast matmuls."""
                lo = blk * 512
                # blk 0: diagonal tiles first (off0 is full-width so start=True
                # covers all 512 cols).  blk>=1: FULL tiles first (also
                # full-width starts) with the diagonals last -- the head's
                # first ot matmuls then only need OLD v_sb tiles, decoupling
                # them from the cycle-boundary vt-copy/transpose chain.
                diags = [4 * blk + off for off in range(4)]
                if blk == 0:
                    seq = diags
                else:
                    seq = list(range(4 * blk)) + diags
                on_pe = (blk == 0)
                ot_ps = ps_ot.tile([128, 512], f32, tag="psOT", name="otps")
                if on_pe:
                    dn_ps = ps_k.tile([128, 512], f32, tag="psK", name="dnps")
                    sumA = sumB = None
                else:
                    dn_ps = None
                    sumA = sum_pool.tile([128, 512], f32r, tag="sumA",
                                         name="sumA")
                    sumB = sum_pool.tile([128, 512], f32r, tag="sumB",
                                         name="sumB")
                nj = len(seq)
                na = nb = 0
                for j, i in enumerate(seq):
                    off = i - 4 * blk
                    col0 = off * 128 if off >= 0 else 0
                    st = (j == 0)
                    sp = (j == nj - 1)
                    pt = ps_x.tile([128, 512], f32, tag="psX", name="pt")
                    nc.tensor.matmul(pt[:, col0:512],
                                     kt[:, i * 128:(i + 1) * 128],
                                     qt[h][:, lo + col0:lo + 512],
                                     start=True, stop=True)
                    et = et_pool.tile([128, 512], bf16, tag="et", name="et")
                    nc.scalar.activation(et[:, col0:512], pt[:, col0:512],
                                         EXP, scale=SCALE)
                    if off >= 0:
                        nc.vector.tensor_mul(et[:, col0:col0 + 128],
                                             et[:, col0:col0 + 128], tri[:])
                    if on_pe:
                        nc.tensor.matmul(dn_ps[:, col0:512], ones[:],
                                         et[:, col0:512], start=st, stop=sp)
                    else:
                        # diagonals + odd fulls on DVE, even fulls on GpSimd
                        # (GpSimd elementwise is ~1.5x slower)
                        to_b = (off < 0) and ((i % 2) == 0)
                        if to_b:
                            eng, s, first = nc.gpsimd, sumB, (nb == 0)
                            nb += 1
                        else:
                            eng, s, first = nc.vector, sumA, (na == 0)
                            na += 1
                        if first:
                            eng.tensor_copy(s[:, col0:512], et[:, col0:512])
                        else:
                            eng.tensor_add(s[:, col0:512], s[:, col0:512],
                                           et[:, col0:512])
                    nc.tensor.matmul(ot_ps[:, col0:512], v_sb[:, i, :],
                                     et[:, col0:512], start=st, stop=sp)
                return ot_ps, dn_ps, sumA, sumB

            def attn_bcast(sumA, sumB):
                """partition-reduce+broadcast the two column-sum tiles."""
                dn_ps = ps_k.tile([128, 512], f32, tag="psK", name="dnps")
                nc.tensor.matmul(dn_ps[:], onesf[:], sumA[:],
                                 start=True, stop=False)
                nc.tensor.matmul(dn_ps[:], onesf[:], sumB[:],
                                 start=False, stop=True)
                return dn_ps

            def attn_epilogue(blk, h, ot_ps, dn_ps):
                lo = blk * 512
                sbc = sbc_pool.tile([128, 512], f32, tag="sbc", name="sbc")
                nc.vector.reciprocal_approx_fast(sbc[:], dn_ps[:])
                nc.vector.tensor_mul(at[h][:, lo:lo + 512], ot_ps[:], sbc[:])

            def oproj_group(blk, g):
                """outT rows [g*512, (g+1)*512) for seq chunk blk (g in 0..7)."""
                lo = blk * 512
                osb = osb_pool.tile([128, 4, 512], bf16, tag="osb", name="osb")
                g4 = g * 4
                for t in range(4):
                    nt = g4 + t
                    o_ps = ps_a.tile([128, 512], f32, tag="psA", name="ops")
                    for h in range(HPC):
                        nc.tensor.matmul(
                            o_ps[:],
                            wo_sb[:, h, nt * 128:(nt + 1) * 128],
                            at[h][:, lo:lo + 512],
                            start=(h == 0), stop=(h == HPC - 1))
                    if t < 2:
                        nc.scalar.activation(osb[:, t, :], o_ps[:], CPY)
                    else:
                        nc.vector.tensor_copy(osb[:, t, :], o_ps[:])
                nc.sync.dma_start(out=out_r[:, g4:g4 + 4, lo:lo + 512],
                                  in_=osb[:])

            # -------- schedule --------
            # o-proj groups of block b-1 are spread across cycle b as PE
            # filler: 2 after proj (cover the RoPE drain into h0), 2 after
            # h0 and h1, 1 after h2 and h3.  Each head's denominator
            # broadcast matmuls go after its filler groups so they never
            # stall the PE waiting on the DVE/GpSimd column sums.
            slot_groups = [(0, 1), (2, 3), (4, 5), (6,), (7,)]
            for blk in range(NBLK):
                vt_sb, pend = proj_block(blk)
                if blk >= 1:
                    for g in slot_groups[0]:
                        oproj_group(blk - 1, g)
                vtrans(vt_sb, blk)
                for h in range(HPC):
                    core = attn_head_core(blk, h)
                    ot_ps, dn_ps, sumA, sumB = core
                    if h < 2:
                        q_sb, q_sw, hh = pend[h]
                        rope_sb(nc.gpsimd, q_sb, q_sw, qt[hh], blk)
                    if blk == 0:
                        attn_epilogue(blk, h, ot_ps, dn_ps)
                    else:
                        for g in slot_groups[1 + h]:
                            oproj_group(blk - 1, g)
                        dn_ps = attn_bcast(sumA, sumB)
                        attn_epilogue(blk, h, ot_ps, dn_ps)
            for g in range(8):
                oproj_group(NBLK - 1, g)

    nc.compile()
    return nc


def _host_prep(hidden_states, position_ids, Wq, Wk, Wv, Wo):
    import ml_dtypes
    bf16 = ml_dtypes.bfloat16

    X = np.asarray(hidden_states, dtype=np.float32).reshape(S, HIDDEN)
    XT = np.ascontiguousarray(X.T).astype(bf16)

    pos = np.asarray(position_ids).reshape(-1)[:S].astype(np.float32)
    inv = (1.0 / (ROPE_BASE ** (np.arange(0, HD, 2, dtype=np.float32) / HD))
           ).astype(np.float32)
    freqs = pos[:, None] * inv[None, :]              # [S, 64]
    cos_h = np.cos(freqs).astype(np.float32)         # [S, 64] (= both halves)
    sin_h = np.sin(freqs).astype(np.float32)
    cosT = np.ascontiguousarray(np.concatenate([cos_h, cos_h], axis=1).T)
    sinT = np.concatenate([sin_h, sin_h], axis=1).T
    sinS = np.ascontiguousarray(np.concatenate([-sinT[0:64], sinT[64:128]],
                                               axis=0))

    tri = (np.arange(128)[:, None] <= np.arange(128)[None, :]).astype(bf16)
    ones = np.ones((128, 128), dtype=bf16)
    onesf = np.ones((128, 128), dtype=np.float32)
    ident = np.eye(128, dtype=np.float32).astype(bf16)

    Wq = np.asarray(Wq, dtype=np.float32)
    Wk = np.asarray(Wk, dtype=np.float32)
    Wv = np.asarray(Wv, dtype=np.float32)
    Wo = np.asarray(Wo, dtype=np.float32)

    in_maps = []
    for c in range(N_CORES):
        in_maps.append({
            "xt": XT,
            "wq": np.ascontiguousarray(Wq[:, c * DQ:(c + 1) * DQ]).astype(bf16),
            "wk": np.ascontiguousarray(Wk[:, c * HD:(c + 1) * HD]).astype(bf16),
            "wv": np.ascontiguousarray(Wv[:, c * HD:(c + 1) * HD]).astype(bf16),
            "wo": np.ascontiguousarray(Wo[c * DQ:(c + 1) * DQ, :]).astype(bf16),
            "cosT": cosT,
            "sinS": sinS,
            "tri": tri,
            "ones": ones,
            "onesf": onesf,
            "ident": ident,
        })
    return in_maps


def kernel(hidden_states, position_ids, Wq, Wk, Wv, Wo, _run_opts=None):
    from concourse.bass_utils import run_bass_kernel_spmd

    if "nc" not in _CACHE:
        _CACHE["nc"] = _build()
    nc = _CACHE["nc"]

    in_maps = _host_prep(hidden_states, position_ids, Wq, Wk, Wv, Wo)
    opts = dict(_run_opts or {})
    res = run_bass_kernel_spmd(nc, in_maps, core_ids=list(range(N_CORES)), **opts)
    _CACHE["last_result"] = res

    outT = np.zeros((HIDDEN, S), dtype=np.float64)
    for c in range(N_CORES):
        outT += res.results[c]["out"].astype(np.float32)
    return outT.T.astype(np.float32).reshape(1, S, HIDDEN)
